# revision 26
# baseline (speedup 1.0000x reference)
import sys

sys.path.insert(0, "/opt/trn_rl_repo")

import numpy as np

import concourse.bacc as bacc
import concourse.bass as bass
import concourse.mybir as mybir
import concourse.tile as tile
from concourse.bass_utils import run_bass_kernel_spmd

# Problem shapes (hardcoded per contract)
B = 4
NQ = 2048
NR = 16384
D = 64
K = 16

NCORES = 8
QPC = NQ // 2          # queries per core (each batch split across 2 cores)
NCHUNK = QPC // 128    # query chunks of 128 per core
MMN = 512              # matmul free dim (one PSUM bank of fp32)
XRAW = 460             # per chunk, one quad ships 2*XRAW cols raw via Act
NWIN = NR // 2 + XRAW  # pooled + raw output columns per query row
TOPW = 32              # windows kept per query on host (slack over K=16)


def _quad_layout(first_chunk: bool):
    """Quads as (ref_start, half_width, x_raw): psA covers refs
    [s, s+h), psB covers [s+h, s+2h). The first h-x output columns are
    width-2 pooled windows (max over refs {s+j, s+h+j}); the hybrid
    quad (x>0) additionally ships its last x columns of each side raw
    (width-1), drained by the Act engine to offload the DVE. First
    chunk leads with two small quads so the drain pipeline starts
    early."""
    if first_chunk:
        quads = [(0, 512, 0), (1024, 512, 0)]
        s = 2048
    else:
        quads = []
        s = 0
    hybrid_done = False
    while s < NR:
        x = 0 if hybrid_done else XRAW
        hybrid_done = True
        quads.append((s, 1024, x))
        s += 2048
    return quads


def _win_maps():
    """Per-layout window->ref maps (A-side and B-side ref index per
    output column; raw columns have wa == wb)."""
    maps = {}
    for first in (True, False):
        wa = np.empty(NWIN, dtype=np.int64)
        wb = np.empty(NWIN, dtype=np.int64)
        col = 0
        for s, h, x in _quad_layout(first):
            hp = h - x
            j = np.arange(hp)
            wa[col:col + hp] = s + j
            wb[col:col + hp] = s + h + j
            col += hp
            if x:
                j = np.arange(x)
                wa[col:col + x] = s + hp + j
                wb[col:col + x] = s + hp + j
                col += x
                wa[col:col + x] = s + h + hp + j
                wb[col:col + x] = s + h + hp + j
                col += x
        assert col == NWIN, (col, NWIN)
        maps[first] = (wa, wb)
    return maps


_WIN_MAPS = _win_maps()
_prog_cache = {}


def _build_program(reps: int = 1):
    if reps in _prog_cache:
        return _prog_cache[reps]

    f32 = mybir.dt.float32
    bf16 = mybir.dt.bfloat16
    mx = mybir.AluOpType.max

    nc = bacc.Bacc("TRN2", target_bir_lowering=False, debug=False, num_devices=NCORES)

    # lhsT rows 0..63 = 2*q^T, row 64 = 1.0; rhs rows 0..63 = r^T,
    # row 64 = -(r2 - mean(r2)). psum = 2qr - r2 + m = -d2 + q2 + m:
    # a per-row constant offset away from -d2, so per-row ranking (all
    # the device is used for) is unaffected. bf16 halves the input DMA.
    lhs_d = nc.dram_tensor("lhs", [65, QPC], bf16, kind="ExternalInput")
    rhs_d = nc.dram_tensor("rhs", [65, NR], bf16, kind="ExternalInput")
    outP_d = nc.dram_tensor("outP", [QPC, NWIN], bf16, kind="ExternalOutput")

    # small leading pieces so the first quads' matmuls start early
    RHS_CUTS = [0, 1024, 2048, 4096, 8192, 12288, 16384]

    with tile.TileContext(nc) as tc:
        with (
            tc.tile_pool(name="consts", bufs=1) as cpool,
            tc.tile_pool(name="psum", bufs=2, space="PSUM") as ppool,
            tc.tile_pool(name="stage", bufs=3) as spool,
            tc.tile_pool(name="outs", bufs=2) as opool,
        ):
            lhs_t = cpool.tile([65, QPC], bf16)
            nc.sync.dma_start(lhs_t[:, 0:128], lhs_d.ap()[:, 0:128])
            rhs_t = cpool.tile([65, NR], bf16)
            nc.sync.dma_start(rhs_t[:, 0:RHS_CUTS[1]], rhs_d.ap()[:, 0:RHS_CUTS[1]])
            nc.sync.dma_start(lhs_t[:, 128:QPC], lhs_d.ap()[:, 128:QPC])
            for p in range(1, len(RHS_CUTS) - 1):
                a, b = RHS_CUTS[p], RHS_CUTS[p + 1]
                nc.sync.dma_start(rhs_t[:, a:b], rhs_d.ap()[:, a:b])

            for rep in range(reps):
              for c in range(NCHUNK):
                lhs_c = lhs_t[:, c * 128:(c + 1) * 128]
                quads = _quad_layout(c == 0)
                nquads = len(quads)
                out = opool.tile([128, NWIN], bf16, tag="out")
                col = 0
                half_col = None
                r0 = c * 128
                last = (rep == reps - 1) and (c == NCHUNK - 1)
                for qi, (s, h, x) in enumerate(quads):
                    hp = h - x
                    psA = ppool.tile([128, 1024], f32, tag="pa")
                    psB = ppool.tile([128, 1024], f32, tag="pb")
                    for m0 in range(0, h, MMN):
                        mw = min(MMN, h - m0)
                        nc.tensor.matmul(
                            psA[:, m0:m0 + mw], lhs_c,
                            rhs_t[:, s + m0:s + m0 + mw],
                            start=True, stop=True,
                        )
                    sA = spool.tile([128, 1024], f32, tag="sa")
                    nc.scalar.copy(sA[:, 0:hp], psA[:, 0:hp])
                    if x:
                        # hybrid quad: Act drains the tails of both sides
                        # straight to raw bf16 output, shortening the TT
                        nc.scalar.copy(out[:, col + hp:col + hp + x], psA[:, hp:h])
                    for m0 in range(0, h, MMN):
                        mw = min(MMN, h - m0)
                        nc.tensor.matmul(
                            psB[:, m0:m0 + mw], lhs_c,
                            rhs_t[:, s + h + m0:s + h + m0 + mw],
                            start=True, stop=True,
                        )
                    final = last and (qi == nquads - 1)
                    if not final:
                        nc.vector.tensor_tensor(
                            out[:, col:col + hp], psB[:, 0:hp], sA[:, 0:hp], mx
                        )
                    else:
                        # split the very last drain so its first half ships
                        # while the second half still runs
                        h2 = hp // 2
                        nc.vector.tensor_tensor(
                            out[:, col:col + h2], psB[:, 0:h2], sA[:, 0:h2], mx
                        )
                        nc.vector.tensor_tensor(
                            out[:, col + h2:col + hp],
                            psB[:, h2:hp], sA[:, h2:hp], mx,
                        )
                    if x:
                        nc.scalar.copy(
                            out[:, col + hp + x:col + hp + 2 * x], psB[:, hp:h]
                        )
                    col0 = col
                    col += hp + 2 * x
                    # stream results out: halves normally; per-quad on the
                    # last chunk so nothing big queues in front of the tail
                    if last:
                        if final:
                            h2 = hp // 2
                            nc.sync.dma_start(
                                outP_d.ap()[r0:r0 + 128, col0:col0 + h2],
                                out[:, col0:col0 + h2],
                            )
                            nc.sync.dma_start(
                                outP_d.ap()[r0:r0 + 128, col0 + h2:col],
                                out[:, col0 + h2:col],
                            )
                        else:
                            nc.sync.dma_start(
                                outP_d.ap()[r0:r0 + 128, col0:col],
                                out[:, col0:col],
                            )
                    elif half_col is None and col >= NWIN // 2:
                        nc.sync.dma_start(
                            outP_d.ap()[r0:r0 + 128, 0:col], out[:, 0:col]
                        )
                        half_col = col
                if not last:
                    nc.sync.dma_start(
                        outP_d.ap()[c * 128:(c + 1) * 128, half_col:NWIN],
                        out[:, half_col:NWIN],
                    )

    nc.compile()
    _prog_cache[reps] = nc
    return nc


def kernel(ref: np.ndarray, query: np.ndarray):
    ref = np.asarray(ref, dtype=np.float32)
    query = np.asarray(query, dtype=np.float32)

    # host-side operand prep (layout + norms)
    r2 = np.sum(ref * ref, axis=-1)                      # [B, NR]
    q2 = np.sum(query * query, axis=-1)                  # [B, NQ]
    refT = np.ascontiguousarray(ref.transpose(0, 2, 1))  # [B, D, NR]
    qT = np.ascontiguousarray(query.transpose(0, 2, 1))  # [B, D, NQ]

    nc = _build_program()

    import ml_dtypes
    bf = ml_dtypes.bfloat16
    in_maps = []
    for core in range(NCORES):
        b, h = core // 2, core % 2
        lhs = np.empty((65, QPC), dtype=np.float32)
        lhs[0:D, :] = 2.0 * qT[b][:, h * QPC:(h + 1) * QPC]
        lhs[D, :] = 1.0
        rhs = np.empty((65, NR), dtype=np.float32)
        rhs[0:D, :] = refT[b]
        rhs[D, :] = -(r2[b] - r2[b].mean())  # centered: small bf16 abs error
        in_maps.append({"lhs": lhs.astype(bf), "rhs": rhs.astype(bf)})

    res = run_bass_kernel_spmd(nc, in_maps, core_ids=list(range(NCORES)))

    # host-side top-k: pick the best TOPW pooled windows per query (pooled
    # values are bf16 maxima of -d2 over ref pairs), expand to 2*TOPW
    # candidate refs, rescore exactly, take the smallest K.
    wa0, wb0 = _WIN_MAPS[True]    # rows 0:128 of each core (chunk 0)
    war, wbr = _WIN_MAPS[False]   # remaining rows
    Dout = np.empty((B, NQ, K), dtype=np.float32)
    Iout = np.empty((B, NQ, K), dtype=np.int64)
    rows = np.arange(NQ)[:, None]
    for b in range(B):
        pooled = np.concatenate(
            [
                np.asarray(res.results[2 * b]["outP"]).astype(np.float32),
                np.asarray(res.results[2 * b + 1]["outP"]).astype(np.float32),
            ],
            axis=0,
        )                                                    # [NQ, NWIN]
        widx = np.argpartition(-pooled, TOPW, axis=1)[:, :TOPW]  # [NQ, TOPW]
        first_chunk = np.zeros(NQ, dtype=bool)
        first_chunk[0:128] = True          # core 2b chunk 0
        first_chunk[QPC:QPC + 128] = True  # core 2b+1 chunk 0
        wa = np.where(first_chunk[:, None], wa0[widx], war[widx])
        wb = np.where(first_chunk[:, None], wb0[widx], wbr[widx])
        cand = np.concatenate([wa, wb], axis=1)              # [NQ, TOPW*2]
        cand.sort(axis=1)                                    # id-order tie-break
        rg = ref[b][cand]                                    # [NQ, TOPW*2, D]
        d2 = (
            q2[b][:, None]
            + r2[b][cand]
            - 2.0 * np.einsum("qd,qkd->qk", query[b], rg, dtype=np.float64)
        )
        d2[:, 1:][cand[:, 1:] == cand[:, :-1]] = np.inf      # raw-window dups
        ordk = np.argsort(d2, axis=1, kind="stable")[:, :K]
        d2k = np.maximum(d2[rows, ordk], 0.0)
        Dout[b] = np.sqrt(d2k).astype(np.float32)
        Iout[b] = cand[rows, ordk]
    return (Dout, Iout)


# revision 35
# speedup vs baseline: 1.0466x; 1.0466x over previous
import sys

sys.path.insert(0, "/opt/trn_rl_repo")

import numpy as np

import concourse.bacc as bacc
import concourse.bass as bass
import concourse.mybir as mybir
import concourse.tile as tile
from concourse.bass_utils import run_bass_kernel_spmd

# Problem shapes (hardcoded per contract)
B = 4
NQ = 2048
NR = 16384
D = 64
K = 16

NCORES = 8
QPC = NQ // 2          # queries per core (each batch split across 2 cores)
NCHUNK = QPC // 128    # query chunks of 128 per core
MMN = 512              # matmul free dim (one PSUM bank of fp32)
QUAD = 2048            # refs per drain quad (4 PSUM banks)
NQUAD = NR // QUAD     # 8 quads per chunk
HQ = QUAD // 2         # 1024 cols per psum operand (2 banks)
NWIN = NR // 2         # 8192 width-2 pooled windows per query row
TOPW = 32              # windows kept per query on host (slack over K=16)

# Window w = t*1024 + j (t = quad, j in [0,1024)) is the bf16 max of
# -d2 over the ref pair {t*2048 + j, t*2048 + 1024 + j}.
_WIN_A = (np.arange(NWIN, dtype=np.int64) >> 10) * QUAD + (
    np.arange(NWIN, dtype=np.int64) & 1023
)
_WIN_B = _WIN_A + HQ

_prog_cache = {}


def _build_program(reps: int = 1):
    if reps in _prog_cache:
        return _prog_cache[reps]

    f32 = mybir.dt.float32
    bf16 = mybir.dt.bfloat16
    mx = mybir.AluOpType.max

    nc = bacc.Bacc("TRN2", target_bir_lowering=False, debug=False, num_devices=NCORES)

    # lhsT rows 0..63 = 2*q^T, row 64 = 1.0; rhs rows 0..63 = r^T,
    # row 64 = -(r2 - mean(r2)). psum = 2qr - r2 + m = -d2 + (q2 + m):
    # a per-row constant offset away from -d2, so per-row ranking (all
    # the device is used for) is unaffected. bf16 halves the input DMA;
    # bf16 matmul runs at the same 1 cycle/row as fp32r.
    lhs_d = nc.dram_tensor("lhs", [65, QPC], bf16, kind="ExternalInput")
    rhs_d = nc.dram_tensor("rhs", [65, NR], bf16, kind="ExternalInput")
    outP_d = nc.dram_tensor("outP", [QPC, NWIN], bf16, kind="ExternalOutput")

    # small leading pieces so the first quad's matmuls start early
    RHS_CUTS = [0, 2048, 4096, 8192, 12288, 16384]

    with tile.TileContext(nc) as tc:
        with (
            tc.tile_pool(name="consts", bufs=1) as cpool,
            tc.tile_pool(name="psum", bufs=2, space="PSUM") as ppool,
            tc.tile_pool(name="stage", bufs=3) as spool,
            tc.tile_pool(name="outs", bufs=2) as opool,
        ):
            lhs_t = cpool.tile([65, QPC], bf16)
            nc.sync.dma_start(lhs_t[:, 0:128], lhs_d.ap()[:, 0:128])
            rhs_t = cpool.tile([65, NR], bf16)
            nc.sync.dma_start(rhs_t[:, 0:RHS_CUTS[1]], rhs_d.ap()[:, 0:RHS_CUTS[1]])
            nc.sync.dma_start(lhs_t[:, 128:QPC], lhs_d.ap()[:, 128:QPC])
            for p in range(1, len(RHS_CUTS) - 1):
                a, b = RHS_CUTS[p], RHS_CUTS[p + 1]
                nc.sync.dma_start(rhs_t[:, a:b], rhs_d.ap()[:, a:b])

            for rep in range(reps):
              for c in range(NCHUNK):
                lhs_c = lhs_t[:, c * 128:(c + 1) * 128]
                out = opool.tile([128, NWIN], bf16, tag="out")
                r0 = c * 128
                last = (rep == reps - 1) and (c == NCHUNK - 1)
                for t in range(NQUAD):
                    t0 = t * QUAD
                    # psA/psB each span two PSUM banks, filled by two
                    # 512-wide bf16 matmuls (1 cycle/row at full p-state)
                    psA = ppool.tile([128, HQ], f32, tag="pa")
                    psB = ppool.tile([128, HQ], f32, tag="pb")
                    for h in range(2):
                        nc.tensor.matmul(
                            psA[:, h * MMN:(h + 1) * MMN], lhs_c,
                            rhs_t[:, t0 + h * MMN:t0 + (h + 1) * MMN],
                            start=True, stop=True,
                        )
                    # drain split: Act copies the A side to SBUF...
                    sA = spool.tile([128, HQ], f32, tag="sa")
                    nc.scalar.copy(sA[:], psA[:])
                    for h in range(2):
                        nc.tensor.matmul(
                            psB[:, h * MMN:(h + 1) * MMN], lhs_c,
                            rhs_t[:, t0 + HQ + h * MMN:t0 + HQ + (h + 1) * MMN],
                            start=True, stop=True,
                        )
                    # ...and one DVE tensor_tensor fuses the B-side drain
                    # with the width-2 max-pool (psum operand + sbuf
                    # operand -> bf16 pooled output)
                    w0 = t * HQ
                    final = last and (t == NQUAD - 1)
                    if not final:
                        nc.vector.tensor_tensor(
                            out[:, w0:w0 + HQ], psB[:], sA[:], mx
                        )
                    else:
                        # split the very last drain so its first half
                        # ships while the second half still runs
                        nc.vector.tensor_tensor(
                            out[:, w0:w0 + MMN], psB[:, 0:MMN], sA[:, 0:MMN], mx
                        )
                        nc.vector.tensor_tensor(
                            out[:, w0 + MMN:w0 + HQ],
                            psB[:, MMN:HQ], sA[:, MMN:HQ], mx,
                        )
                    # stream results out: halves normally; per-quad on the
                    # last chunk so nothing big queues in front of the tail
                    if last:
                        if final:
                            nc.sync.dma_start(
                                outP_d.ap()[r0:r0 + 128, w0:w0 + MMN],
                                out[:, w0:w0 + MMN],
                            )
                            nc.sync.dma_start(
                                outP_d.ap()[r0:r0 + 128, w0 + MMN:w0 + HQ],
                                out[:, w0 + MMN:w0 + HQ],
                            )
                        else:
                            nc.sync.dma_start(
                                outP_d.ap()[r0:r0 + 128, w0:w0 + HQ],
                                out[:, w0:w0 + HQ],
                            )
                    elif t == NQUAD // 2 - 1:
                        nc.sync.dma_start(
                            outP_d.ap()[r0:r0 + 128, 0:NWIN // 2],
                            out[:, 0:NWIN // 2],
                        )
                if not last:
                    nc.sync.dma_start(
                        outP_d.ap()[r0:r0 + 128, NWIN // 2:NWIN],
                        out[:, NWIN // 2:NWIN],
                    )

    nc.compile()
    _prog_cache[reps] = nc
    return nc


def kernel(ref: np.ndarray, query: np.ndarray):
    ref = np.asarray(ref, dtype=np.float32)
    query = np.asarray(query, dtype=np.float32)

    # host-side operand prep (layout + norms)
    r2 = np.sum(ref * ref, axis=-1)                      # [B, NR]
    q2 = np.sum(query * query, axis=-1)                  # [B, NQ]
    refT = np.ascontiguousarray(ref.transpose(0, 2, 1))  # [B, D, NR]
    qT = np.ascontiguousarray(query.transpose(0, 2, 1))  # [B, D, NQ]

    nc = _build_program()

    import ml_dtypes
    bf = ml_dtypes.bfloat16
    in_maps = []
    for core in range(NCORES):
        b, h = core // 2, core % 2
        lhs = np.empty((65, QPC), dtype=np.float32)
        lhs[0:D, :] = 2.0 * qT[b][:, h * QPC:(h + 1) * QPC]
        lhs[D, :] = 1.0
        rhs = np.empty((65, NR), dtype=np.float32)
        rhs[0:D, :] = refT[b]
        rhs[D, :] = -(r2[b] - r2[b].mean())  # centered: small bf16 abs error
        in_maps.append({"lhs": lhs.astype(bf), "rhs": rhs.astype(bf)})

    res = run_bass_kernel_spmd(nc, in_maps, core_ids=list(range(NCORES)))

    # host-side top-k: pick the best TOPW pooled windows per query, expand
    # to 2*TOPW candidate refs, rescore exactly, take the smallest K.
    # Correctness: window(w) value >= -d2 of both its members, so every
    # true top-16 ref lives in a top-16 window (tournament argument);
    # TOPW = 32 absorbs bf16 rounding of the pooled values.
    Dout = np.empty((B, NQ, K), dtype=np.float32)
    Iout = np.empty((B, NQ, K), dtype=np.int64)
    rows = np.arange(NQ)[:, None]
    for b in range(B):
        pooled = np.concatenate(
            [
                np.asarray(res.results[2 * b]["outP"]).astype(np.float32),
                np.asarray(res.results[2 * b + 1]["outP"]).astype(np.float32),
            ],
            axis=0,
        )                                                    # [NQ, NWIN]
        widx = np.argpartition(-pooled, TOPW, axis=1)[:, :TOPW]  # [NQ, TOPW]
        cand = np.concatenate([_WIN_A[widx], _WIN_B[widx]], axis=1)
        cand.sort(axis=1)                                    # id-order tie-break
        rg = ref[b][cand]                                    # [NQ, TOPW*2, D]
        d2 = (
            q2[b][:, None]
            + r2[b][cand]
            - 2.0 * np.einsum("qd,qkd->qk", query[b], rg, dtype=np.float64)
        )
        ordk = np.argsort(d2, axis=1, kind="stable")[:, :K]
        d2k = np.maximum(d2[rows, ordk], 0.0)
        Dout[b] = np.sqrt(d2k).astype(np.float32)
        Iout[b] = cand[rows, ordk]
    return (Dout, Iout)
